# revision 34
# baseline (speedup 1.0000x reference)
"""Trainium2 Bass kernel for nn_AggregationLayer (smooth and/or fold over 64 columns).

Math (default VARIANT="psum", the shift-softplus form):
  probs = softmax(selection_weights, axis=1)           # [63, 2]
  s_0 = x[:, 0]
  step i: s' = p0*smoothmin(s, x_i) + p1*smoothmax(s, x_i)   (sharpness 10)
Rewritten on the delta state D = s - x_next with per-step rescale abar
(c_i = max(p0,p1) >= 1/2, sigma_i = sign(p0-p1), abar_i = abar_{i-1}/c_i):
  D' = c*D + gamma*softplus(sigma*10*D) + Delta
  softplus(t) = ln(exp(t - C) + e^-C) + C     exactly, for |t| <= 65.7 <= C+44
(the Ln LUT is only accurate on inputs in [e^-44.6, e^44.6]; C=28 centers the
needed window). Scaled so the state coefficient is exactly 1:
  Dhat' = Dhat + gtil_i * L + dhat_i
  L = Ln(Exp(scale_i * Dhat))   # both on ACT; exp bias -C and the +gamma*C,
                                # exp-shift, and Delta terms all folded into
                                # the host-precomputed dhat_i rows.
Per step per stream: 2 ACT LUT ops (exp, ln) + 2 DVE ops (off-chain
tensor_add state+dhat, on-chain scalar_tensor_tensor gtil*L + tmp).
Two row-streams per core (F=492 each) keep ACT/DVE ~95% busy; the off-chain
add plus stream staggering hides the chain latency.

Distribution: pure data parallel over rows; 8 cores x 125,952 rows (last core
zero-padded). Host pre-transposes x, converts to scaled deltas, and folds all
per-step affine constants into the dhat rows (dh[0] is the initial state).
"""

import contextlib
import ctypes
import sys
import types

import numpy as np

P = 128          # SBUF partitions
F = 246          # rows per partition per tile
T = 4            # tiles per core
N_CORES = 8
RC = P * F * T   # 125,952 rows per core
N_ROWS = 1_000_000
N_COL = 64
N_STEP = 63

_CACHE = {}
TRACE = False
LAST = {}


# ---------------------------------------------------------------- axon NTFF shim
def _ensure_ntff_hook():
    """Provide antenv.axon_hooks (NTFF profiling) if the image lacks it."""
    try:
        from antenv.axon_hooks import get_axon_ntff_profile_hook  # noqa: F401
        return
    except ImportError:
        pass

    so_path = "/opt/axon/libaxon_pjrt.so"
    try:
        lib = ctypes.CDLL(so_path)
    except OSError:
        return
    if not hasattr(lib, "axon_start_nrt_profile"):
        return
    lib.axon_start_nrt_profile.argtypes = [ctypes.POINTER(ctypes.c_int64), ctypes.c_size_t]
    lib.axon_start_nrt_profile.restype = ctypes.c_int64
    lib.axon_stop_nrt_profile.argtypes = [ctypes.c_char_p]
    lib.axon_stop_nrt_profile.restype = ctypes.c_int64

    @contextlib.contextmanager
    def _hook(output_dir, device_ids):
        import jax

        jax.devices()
        if device_ids:
            ids = (ctypes.c_int64 * len(device_ids))(*device_ids)
            rc = lib.axon_start_nrt_profile(ids, len(device_ids))
        else:
            rc = lib.axon_start_nrt_profile(None, 0)
        if rc != 0:
            raise RuntimeError(f"axon_start_nrt_profile rc={rc}")
        try:
            yield
        finally:
            n = lib.axon_stop_nrt_profile(str(output_dir).encode())
            print(f"profile: {n} file(s) written to {output_dir}", file=sys.stderr)

    mod = types.ModuleType("antenv.axon_hooks")
    mod.get_axon_ntff_profile_hook = lambda: _hook
    mod.set_axon_ntff_profile_hook = lambda h: None
    sys.modules["antenv.axon_hooks"] = mod


# ---------------------------------------------------------------- device program
def _patch_act_tables(bacc, mybir):
    """Pin exp/ln/abs to the one set containing all three.

    Bacc's act-table-load pass assigns each activation the FIRST set that
    contains its function (exp -> exp_and_others, ln -> natural_log), which
    makes exp..ln sequences reload tables on every step (~2.6us each).
    Removing those funcs from every set except natural_log_exp_and_others
    (indices preserved) forces a single resident table set.
    """
    if getattr(bacc, "_act_tables_patched", False):
        return
    import concourse.hw_specs as hw_specs

    orig = hw_specs.get_activation_tables
    AF = mybir.ActivationFunctionType
    pinned = {AF.Exp, AF.Ln, AF.Abs}

    def patched(module_arch):
        tables = dict(orig(module_arch))
        out = {}
        for name, funcs in tables.items():
            if name == "natural_log_exp_and_others":
                out[name] = funcs
            else:
                out[name] = funcs - pinned
        return out

    bacc.get_activation_tables = patched
    bacc._act_tables_patched = True


SHIFT_C = 28.0            # exp-domain shift: exp(10d - C), ln(. + e^-C) + C
                          # data max |10D| = 65.7; Ln LUT valid on [e^-44.6, e^44.6]
VARIANT = "psum"          # "delta" (baseline) or "psum" (PSUM-state shift)

# --- psum-variant knobs
import os as _os
PSUM_F = int(_os.environ.get("PSUM_F", 492))       # free dim per tile (F*T == 984)
PSUM_T = int(_os.environ.get("PSUM_T", 2))         # concurrent row streams per core
PSUM_COMBINE = _os.environ.get("PSUM_COMBINE", "dve2")  # "sttmm"|"mm2"|"mix"|"dve2"
PSUM_PREFETCH = int(_os.environ.get("PSUM_PREFETCH", 6))
TT_ENGINE = _os.environ.get("TT_ENGINE", "dve")      # off-chain add engine
SHIFT_C = float(_os.environ.get("SHIFT_C", SHIFT_C))
VARIANT = _os.environ.get("KVARIANT", VARIANT)


def _build_nc(F=F, T=T, variant=None, dma_chunk=8, skew=1, tmp_bufs=3, sp_bufs=4):
    import concourse.bacc as bacc
    import concourse.mybir as mybir
    import concourse.tile as tile
    import math

    _patch_act_tables(bacc, mybir)
    variant = variant or VARIANT

    AF = mybir.ActivationFunctionType
    OP = mybir.AluOpType
    f32 = mybir.dt.float32
    RC = P * F * T

    nc = bacc.Bacc(None)
    xT = nc.dram_tensor("xt", [N_COL, RC], f32, kind="ExternalInput")
    # consts: cols 0..62 = gamma_i, 63..125 = p0_i, 126 = -C, 127 = exp(-C)
    gD = nc.dram_tensor("g", [P, 2 * N_STEP + 2], f32, kind="ExternalInput")
    w2D = (nc.dram_tensor("w2", [P, 2 * P], f32, kind="ExternalInput")
           if variant == "pe" else None)
    outD = nc.dram_tensor("y", [RC], f32, kind="ExternalOutput")

    with tile.TileContext(nc) as tc:
        with (
            tc.tile_pool(name="xp", bufs=3) as xp,
            tc.tile_pool(name="tmp", bufs=tmp_bufs) as tmp,
            tc.tile_pool(name="sp", bufs=sp_bufs) as sp,
            tc.tile_pool(name="gp", bufs=1) as gp,
            tc.tile_pool(name="pp", bufs=4, space="PSUM") as pp,
        ):
            g_sb = gp.tile([P, 2 * N_STEP + 2], f32)
            nc.sync.dma_start(out=g_sb[:], in_=gD[:])
            if variant == "pe":
                w2_sb = gp.tile([P, 2 * P], f32)
                nc.sync.dma_start(out=w2_sb[:], in_=w2D[:])

            def gamma_ap(i):
                return g_sb[:, i - 1 : i]

            def p0_ap(i):
                return g_sb[:, N_STEP + i - 1 : N_STEP + i]

            def emit_step_safe(s_prev, xi, i):
                d = tmp.tile([P, F], f32, tag="d")
                nc.vector.tensor_sub(d[:], s_prev, xi)
                a = tmp.tile([P, F], f32, tag="a")
                nc.scalar.activation(a[:], d[:], AF.Abs, scale=-10.0)
                E = tmp.tile([P, F], f32, tag="E")
                nc.scalar.activation(E[:], a[:], AF.Exp, scale=-1.0)
                L = tmp.tile([P, F], f32, tag="L")
                nc.scalar.activation(L[:], E[:], AF.Ln, bias=1.0)
                u = tmp.tile([P, F], f32, tag="u")
                nc.vector.scalar_tensor_tensor(u[:], d[:], 0.5, xi, OP.mult, OP.add)
                w = tmp.tile([P, F], f32, tag="w")
                nc.vector.scalar_tensor_tensor(w[:], a[:], 0.5, L[:], OP.mult, OP.add)
                s_new = sp.tile([P, F], f32, tag="s")
                nc.vector.scalar_tensor_tensor(
                    s_new[:], w[:], gamma_ap(i), u[:], OP.mult, OP.add
                )
                return s_new[:]

            def emit_step_dvabs(s_prev, xi, i):
                # abs on DVE (tensor_scalar, 2x fp32) -> ACT chain is exp,ln only
                d = tmp.tile([P, F], f32, tag="d")
                nc.vector.tensor_sub(d[:], s_prev, xi)
                a = tmp.tile([P, F], f32, tag="a")
                i32 = mybir.dt.int32
                nc.vector.tensor_scalar(
                    out=a[:].bitcast(i32), in0=d[:].bitcast(i32),
                    scalar1=0x7FFFFFFF, scalar2=0, op0=OP.bitwise_and, op1=OP.bitwise_or,
                )  # a = |d| (sign bit cleared)
                E = tmp.tile([P, F], f32, tag="E")
                nc.scalar.activation(E[:], a[:], AF.Exp, scale=-10.0)
                L = tmp.tile([P, F], f32, tag="L")
                nc.scalar.activation(L[:], E[:], AF.Ln, bias=1.0)
                u = tmp.tile([P, F], f32, tag="u")
                nc.vector.scalar_tensor_tensor(u[:], d[:], 0.5, xi, OP.mult, OP.add)
                w = tmp.tile([P, F], f32, tag="w")
                nc.vector.scalar_tensor_tensor(w[:], a[:], 5.0, L[:], OP.mult, OP.add)
                s_new = sp.tile([P, F], f32, tag="s")
                nc.vector.scalar_tensor_tensor(
                    s_new[:], w[:], gamma_ap(i), u[:], OP.mult, OP.add
                )
                return s_new[:]

            from concourse.tile_rust import add_dep_helper

            delta_ops = {}  # tile id -> list of (chunk_idx, mybir inst)

            def emit_step_delta(s_prev, xi, i, abs_on_act=False, t=None):
                # state D = s - x_next; xi holds Delta_i (or x_63 on the last step)
                # D' = 0.5*D + 5*gamma*|D| + gamma*L(|D|) + Delta_i
                a = tmp.tile([P, F], f32, tag="a")
                if abs_on_act:
                    a_inst = nc.scalar.activation(a[:], s_prev, AF.Abs)  # a = |D|
                else:
                    i32 = mybir.dt.int32
                    a_inst = nc.vector.tensor_scalar(
                        out=a[:].bitcast(i32), in0=s_prev.bitcast(i32),
                        scalar1=0x7FFFFFFF, scalar2=0, op0=OP.bitwise_and, op1=OP.bitwise_or,
                    )  # a = |D|
                E = tmp.tile([P, F], f32, tag="E")
                nc.scalar.activation(E[:], a[:], AF.Exp, scale=-10.0)
                L = tmp.tile([P, F], f32, tag="L")
                nc.scalar.activation(L[:], E[:], AF.Ln, bias=1.0)
                u = tmp.tile([P, F], f32, tag="u")
                u_inst = nc.vector.scalar_tensor_tensor(
                    u[:], s_prev, 0.5, xi, OP.mult, OP.add
                )
                # The in-place x->Delta rewrite is invisible to Tile's dep
                # tracker (out aliases in0); pin the ordering explicitly.
                if t is not None and t in delta_ops:
                    for lo, hi, dinst in delta_ops[t]:
                        if i == 1 and lo == 0:
                            add_dep_helper(a_inst.ins, dinst,
                                           reason="step1 reads Delta_0 state")
                        if i < N_COL - 1 and lo <= i < hi:
                            add_dep_helper(u_inst.ins, dinst,
                                           reason="step reads Delta col")
                w = tmp.tile([P, F], f32, tag="w")
                nc.vector.scalar_tensor_tensor(w[:], a[:], 5.0, L[:], OP.mult, OP.add)
                s_new = sp.tile([P, F], f32, tag="s")
                nc.vector.scalar_tensor_tensor(
                    s_new[:], w[:], gamma_ap(i), u[:], OP.mult, OP.add
                )
                return s_new[:]

            def emit_step_shift(s_prev, xi, i):
                # sp(10d) = ln(exp(10d - C) + e^-C) + C  (exact; no abs needed)
                # s' = p0*d + x_i + gamma*sp(10d)
                d = tmp.tile([P, F], f32, tag="d")
                nc.vector.tensor_sub(d[:], s_prev, xi)
                E = tmp.tile([P, F], f32, tag="E")
                nc.scalar.activation(
                    E[:], d[:], AF.Exp, scale=10.0, bias=g_sb[:, 126:127]
                )
                L = tmp.tile([P, F], f32, tag="L")
                nc.scalar.activation(L[:], E[:], AF.Ln, bias=g_sb[:, 127:128])
                u = tmp.tile([P, F], f32, tag="u")
                nc.vector.scalar_tensor_tensor(u[:], d[:], p0_ap(i), xi, OP.mult, OP.add)
                Lp = tmp.tile([P, F], f32, tag="Lp")
                nc.vector.tensor_scalar(
                    out=Lp[:], in0=L[:], scalar1=SHIFT_C, scalar2=gamma_ap(i),
                    op0=OP.add, op1=OP.mult,
                )
                s_new = sp.tile([P, F], f32, tag="s")
                nc.vector.tensor_add(s_new[:], Lp[:], u[:])
                return s_new[:]

            def emit_step_pe(s_prev, xi, i):
                # u = 0.5*D + Delta_i on the TensorEngine (identity matmuls
                # accumulating in PSUM); DVE does a, w, s' only.
                a = tmp.tile([P, F], f32, tag="a")
                i32 = mybir.dt.int32
                nc.vector.tensor_scalar(
                    out=a[:].bitcast(i32), in0=s_prev.bitcast(i32),
                    scalar1=0x7FFFFFFF, scalar2=0, op0=OP.bitwise_and, op1=OP.bitwise_or,
                )
                E = tmp.tile([P, F], f32, tag="E")
                nc.scalar.activation(E[:], a[:], AF.Exp, scale=-10.0)
                L = tmp.tile([P, F], f32, tag="L")
                nc.scalar.activation(L[:], E[:], AF.Ln, bias=1.0)
                u_ps = pp.tile([P, F], f32, tag="ups")
                nc.tensor.matmul(u_ps[:], w2_sb[:, 0:P], s_prev, start=True, stop=False)
                nc.tensor.matmul(u_ps[:], w2_sb[:, P : 2 * P], xi, start=False, stop=True)
                w = tmp.tile([P, F], f32, tag="w")
                nc.vector.scalar_tensor_tensor(w[:], a[:], 5.0, L[:], OP.mult, OP.add)
                s_new = sp.tile([P, F], f32, tag="s")
                nc.vector.scalar_tensor_tensor(
                    s_new[:], w[:], gamma_ap(i), u_ps[:], OP.mult, OP.add
                )
                return s_new[:]

            emit_step = {"shift": emit_step_shift, "safe": emit_step_safe,
                         "dvabs": emit_step_dvabs, "delta": emit_step_delta,
                         "delta_actabs": (lambda s_, x_, i_, t=None: emit_step_delta(s_, x_, i_, True, t=t)),
                         "delta_hyb": (lambda s_, x_, i_, t=None: emit_step_delta(s_, x_, i_, (t is not None and t % 2 == 0), t=t)),
                         "pe": emit_step_pe}[variant]

            DMA_CHUNK = dma_chunk
            SKEW = skew

            def emit_out(t, s_fin):
                dst = outD[t * P * F : (t + 1) * P * F].rearrange("(p j) -> p j", p=P)
                nc.sync.dma_start(out=dst, in_=s_fin)

            for pair in range(T // 2):
                tiles = [2 * pair, 2 * pair + 1]
                xt = {}
                for t in tiles:
                    xt[t] = xp.tile([P, N_COL, F], f32, tag="xt", name=f"xt{t}")
                    base = t * P * F
                    for c in range(0, N_COL, DMA_CHUNK):
                        src = xT[
                            c : c + DMA_CHUNK, base : base + P * F
                        ].rearrange("i (p j) -> p i j", p=P)
                        nc.sync.dma_start(out=xt[t][:, c : c + DMA_CHUNK, :], in_=src)
                    if variant.startswith("delta"):
                        # in-place x -> Delta: x[:, i, :] -= x[:, i+1, :] for i<63.
                        # Writes trail all reads of each position (reads of pos p
                        # occur at elements <= p*F, the write at p*F + pipe lat).
                        delta_ops[t] = []
                        for c in range(0, N_STEP, DMA_CHUNK):
                            hi = min(c + DMA_CHUNK, N_STEP)
                            dinst = nc.vector.tensor_sub(
                                xt[t][:, c:hi, :],
                                xt[t][:, c:hi, :],
                                xt[t][:, c + 1 : hi + 1, :],
                            )
                            delta_ops[t].append((c, hi, dinst.ins))

                tA, tB = tiles
                s_ap = {t: xt[t][:, 0, :] for t in tiles}
                for i in range(1, N_STEP + SKEW + 2):
                    if i <= N_STEP:
                        s_ap[tA] = emit_step(s_ap[tA], xt[tA][:, i, :], i,
                                             **({"t": tA} if variant.startswith("delta") else {}))
                    elif i == N_STEP + 1:
                        emit_out(tA, s_ap[tA])
                    j = i - SKEW
                    if 1 <= j <= N_STEP:
                        s_ap[tB] = emit_step(s_ap[tB], xt[tB][:, j, :], j,
                                             **({"t": tB} if variant.startswith("delta") else {}))
                    elif j == N_STEP + 1:
                        emit_out(tB, s_ap[tB])

    nc.finalize()
    return nc


def _build_nc_psum(F=None, T=None, combine=None, prefetch=None):
    """PSUM-state kernel: per step i (1..63)
         E = Exp(scale_i * Dhat)          [ACT, psum -> psum, no bias]
         L = Ln(E + e^-C)                 [ACT, psum -> sbuf, const-col bias]
         t = gtil_i * L + dhat_i          [DVE stt]  (sttmm)
         Dhat += t                        [PE eye-matmul accumulate]
       State Dhat lives in PSUM; all per-step affine constants folded into
       host-precomputed dhat rows (delta + gamma*C + exp-shift terms)."""
    import concourse.bacc as bacc
    import concourse.mybir as mybir
    import concourse.tile as tile

    F = F or PSUM_F
    T = T or PSUM_T
    combine = combine or PSUM_COMBINE
    prefetch = prefetch or PSUM_PREFETCH

    _patch_act_tables(bacc, mybir)

    AF = mybir.ActivationFunctionType
    OP = mybir.AluOpType
    f32 = mybir.dt.float32
    RC = P * F * T

    nc = bacc.Bacc(None)
    xT = nc.dram_tensor("xt", [N_COL, RC], f32, kind="ExternalInput")
    # g cols: 0 = e^-C (ln bias), 1 = 1/abar63, 2+i = gtil_{i+1} (i=0..62)
    gD = nc.dram_tensor("g", [P, 2 + N_STEP], f32, kind="ExternalInput")
    # w cols: [eye | gtil_1*eye | ... | gtil_63*eye] (only eye used by sttmm)
    NW = P * (1 + (N_STEP if combine in ("mm2", "mix") else 0))
    wD = nc.dram_tensor("w", [P, NW], f32, kind="ExternalInput")
    outD = nc.dram_tensor("y", [RC], f32, kind="ExternalOutput")

    # host fills via _CACHE["psum_consts"]; scales are python floats baked
    # into the instruction stream, so the build needs them up front.
    scales = _CACHE["psum_scales"]  # [63] python floats, step i -> scales[i-1]
    gtils = _CACHE["psum_gtil"]     # [63] python floats (stt immediates)
    inv_a63 = _CACHE["psum_inv_a63"]

    with tile.TileContext(nc) as tc:
        with (
            tc.tile_pool(name="xp", bufs=prefetch) as xp,
            tc.tile_pool(name="lp", bufs=3) as lp,
            tc.tile_pool(name="tp", bufs=3) as tp,
            tc.tile_pool(name="sp2", bufs=3) as sp2,
            tc.tile_pool(name="op", bufs=2) as op_,
            tc.tile_pool(name="gp", bufs=1) as gp,
            tc.tile_pool(name="stp", bufs=1, space="PSUM") as stp,
            tc.tile_pool(name="ep", bufs=2, space="PSUM") as ep,
        ):
            def fetch(t, i):
                tl = xp.tile([P, F], f32, tag=f"x{t}", name=f"x{t}_{i}")
                base = t * P * F
                src = xT[i, base : base + P * F].rearrange("(p j) -> p j", p=P)
                # split the ramp burst across both HWDGE queues; ACT's
                # queue is idle until the pipeline fills
                eng = nc.scalar if (t % 2 == 1 and i <= prefetch) else nc.sync
                eng.dma_start(out=tl[:], in_=src)
                return tl

            g_sb = gp.tile([P, 2 + N_STEP], f32)
            w_sb = gp.tile([P, NW], f32)

            def gw(i):  # gtil_i * eye   (mm2 only)
                return w_sb[:, i * P : (i + 1) * P]

            def gcol(j):  # gtil for step j+1 as [P,1] (g cols 2..64)
                return g_sb[:, 2 + j : 3 + j]

            # stream modes: "dve" = SBUF state, DVE combine (tt off-chain +
            # stt on-chain); "mm2"/"sttmm" = PSUM state, PE accumulate.
            if combine == "mix":
                modes = ["dve" if t % 2 == 0 else "mm2" for t in range(T)]
            elif combine == "dve2":
                modes = ["dve"] * T
            else:
                modes = [combine] * T

            # x0 fetches first (they gate the first exp), then consts;
            # the eye/weight tile is only needed when the PE is used
            x0s = {t: fetch(t, 0) for t in range(T)}
            nc.sync.dma_start(out=g_sb[:], in_=gD[:])
            eye = None
            if any(m != "dve" for m in modes):
                nc.sync.dma_start(out=w_sb[:], in_=wD[:])
                eye = w_sb[:, 0:P]

            state = {}
            for t in range(T):
                if modes[t] == "dve":
                    state[t] = x0s[t][:]  # Δ̂ row 0 IS the initial state
                else:
                    st = stp.tile([P, F], f32, name=f"st{t}")
                    nc.tensor.matmul(st[:], eye, x0s[t][:], start=True,
                                     stop=True, skip_group_check=True)
                    state[t] = st[:]

            for i in range(1, N_STEP + 1):
                for t in range(T):
                    E = ep.tile([P, F], f32, tag=f"E{t}")
                    nc.scalar.activation(E[:], state[t], AF.Exp,
                                         scale=float(scales[i - 1]))
                    xti = fetch(t, i)
                    if modes[t] == "dve":
                        # off-chain: tmp = state + Δ̂_i (pure read of state)
                        if TT_ENGINE == "dma":
                            # in-place inline add on the fetched Δ̂ tile
                            # (accum DMA requires the SWDGE/gpsimd path)
                            nc.gpsimd.dma_start(out=xti[:], in_=state[t],
                                                accum_op=OP.add)
                            tmp = xti
                        else:
                            tmp = tp.tile([P, F], f32, tag=f"tmp{t}")
                            eng = nc.gpsimd if TT_ENGINE == "gpsimd" else nc.vector
                            eng.tensor_add(tmp[:], state[t], xti[:])
                    elif modes[t] == "mm2":
                        nc.tensor.matmul(state[t], eye, xti[:], start=False,
                                         stop=True, skip_group_check=True)
                    L = lp.tile([P, F], f32, tag=f"L{t}")
                    nc.scalar.activation(L[:], E[:], AF.Ln, bias=g_sb[:, 0:1])
                    if modes[t] == "dve":
                        ns = sp2.tile([P, F], f32, tag=f"s{t}")
                        nc.vector.scalar_tensor_tensor(
                            ns[:], L[:], float(gtils[i - 1]), tmp[:],
                            OP.mult, OP.add
                        )
                        state[t] = ns[:]
                    elif modes[t] == "mm2":
                        nc.tensor.matmul(state[t], gw(i), L[:], start=False,
                                         stop=True, skip_group_check=True)
                    else:  # sttmm
                        tt = tp.tile([P, F], f32, tag=f"t{t}")
                        nc.vector.scalar_tensor_tensor(
                            tt[:], L[:], gcol(i - 1), xti[:], OP.mult, OP.add
                        )
                        nc.tensor.matmul(state[t], eye, tt[:], start=False,
                                         stop=True, skip_group_check=True)

            for t in range(T):
                dst = outD[t * P * F : (t + 1) * P * F].rearrange(
                    "(p j) -> p j", p=P)
                if abs(inv_a63 - 1.0) < 1e-6:
                    nc.sync.dma_start(out=dst, in_=state[t])
                else:
                    o = op_.tile([P, F], f32, tag="o")
                    nc.vector.tensor_scalar(out=o[:], in0=state[t],
                                            scalar1=float(inv_a63),
                                            scalar2=None, op0=OP.mult)
                    nc.sync.dma_start(out=dst, in_=o[:])

    nc.finalize()
    return nc


def _psum_host_prep(x, selection_weights, F, T):
    """Fold softmax, state rescaling, exp-shift and gamma*C constants into
    per-step dhat rows; returns (xt_shards, g_arr, w_arr, inv_a63)."""
    RC = P * F * T
    C = SHIFT_C
    w64 = selection_weights.astype(np.float64)
    e = np.exp(w64 - w64.max(axis=1, keepdims=True))
    p = e / e.sum(axis=1, keepdims=True)
    p0, p1 = p[:, 0], p[:, 1]          # [63], step i uses index i-1
    gamma = (p1 - p0) / 10.0
    c = np.maximum(p0, p1)
    sigma = np.where(p0 >= p1, 1.0, -1.0)

    abar = np.empty(64, dtype=np.float64)
    abar[0] = np.prod(c)               # => abar[63] == 1: state IS the output
    for i in range(1, 64):
        abar[i] = abar[i - 1] / c[i - 1]

    scales = sigma * 10.0 / abar[:63]              # step i -> scales[i-1]
    gtil = abar[1:] * gamma                        # step i -> gtil[i-1]
    shift = np.zeros(65, dtype=np.float64)         # shift[i], i=1..63; 64 -> 0
    shift[1:64] = sigma * abar[:63] * C / 10.0

    _CACHE["psum_scales"] = [float(s) for s in scales]
    _CACHE["psum_gtil"] = [float(g) for g in gtil]
    _CACHE["psum_inv_a63"] = float(1.0 / abar[63])

    # dhat rows on the transposed view
    x = np.asarray(x, dtype=np.float32)
    xT = x.T                                        # [64, N] view
    dh = np.empty((N_COL, N_ROWS), dtype=np.float32)
    # row 0: abar0*(x0-x1) - shift_1
    dh[0] = (xT[0] - xT[1]) * np.float32(abar[0])
    dh[0] += np.float32(-shift[1])
    for i in range(1, 63):
        np.subtract(xT[i], xT[i + 1], out=dh[i])
        dh[i] *= np.float32(abar[i])
        dh[i] += np.float32(abar[i] * gamma[i - 1] * C + shift[i] - shift[i + 1])
    dh[63] = xT[63] * np.float32(abar[63])
    dh[63] += np.float32(abar[63] * gamma[62] * C + shift[63])

    gcols = np.zeros(2 + N_STEP, dtype=np.float32)
    gcols[0] = np.exp(-C)
    gcols[1] = 1.0 / abar[63]
    gcols[2:] = gtil.astype(np.float32)
    g_arr = np.ascontiguousarray(np.broadcast_to(gcols[None, :], (P, 2 + N_STEP)))

    if PSUM_COMBINE in ("mm2", "mix"):
        w_arr = np.zeros((P, P * 64), dtype=np.float32)
        w_arr[:, :P] = np.eye(P, dtype=np.float32)
        for i in range(1, 64):
            w_arr[:, i * P : (i + 1) * P] = np.eye(P) * np.float32(gtil[i - 1])
    else:
        w_arr = np.ascontiguousarray(np.eye(P, dtype=np.float32))

    shards = []
    for k in range(N_CORES):
        sl = dh[:, k * RC : min((k + 1) * RC, N_ROWS)]
        if sl.shape[1] < RC:
            pad = np.zeros((N_COL, RC), np.float32)
            pad[:, : sl.shape[1]] = sl
            sl = pad
        else:
            sl = np.ascontiguousarray(sl)
        shards.append(sl)
    return shards, g_arr, w_arr


def _get_nc():
    if "nc" not in _CACHE:
        _CACHE["nc"] = _build_nc()
    return _CACHE["nc"]


# ---------------------------------------------------------------- host wrapper
def _kernel_psum(x: np.ndarray, selection_weights: np.ndarray) -> np.ndarray:
    _ensure_ntff_hook()
    from concourse.bass_utils import run_bass_kernel_spmd

    F, T = PSUM_F, PSUM_T
    RC = P * F * T
    shards, g_arr, w_arr = _psum_host_prep(x, selection_weights, F, T)
    key = ("psum", F, T, PSUM_COMBINE)
    if _CACHE.get("nc_key") != key:
        _CACHE["nc_psum"] = _build_nc_psum(F=F, T=T)
        _CACHE["nc_key"] = key
    nc = _CACHE["nc_psum"]

    in_maps = [{"xt": shards[k], "g": g_arr, "w": w_arr} for k in range(N_CORES)]
    res = run_bass_kernel_spmd(nc, in_maps, list(range(N_CORES)), trace=TRACE)
    LAST["exec_time_ns"] = getattr(res, "exec_time_ns", None)
    LAST["profile_json"] = getattr(res, "profile_json", None)

    out = np.concatenate([res.results[k]["y"] for k in range(N_CORES)])
    return out[:N_ROWS].reshape(N_ROWS, 1)


def kernel(x: np.ndarray, selection_weights: np.ndarray) -> np.ndarray:
    if VARIANT == "psum":
        return _kernel_psum(x, selection_weights)
    _ensure_ntff_hook()
    from concourse.bass_utils import run_bass_kernel_spmd

    nc = _get_nc()

    # softmax over the (and, or) pair, in float64 for clean constants
    w64 = selection_weights.astype(np.float64)
    e = np.exp(w64 - w64.max(axis=1, keepdims=True))
    p = e / e.sum(axis=1, keepdims=True)
    gamma = ((p[:, 1] - p[:, 0]) / 10.0).astype(np.float32)  # [63]
    p0 = p[:, 0].astype(np.float32)                          # [63]
    extra = np.array([-SHIFT_C, np.exp(-SHIFT_C)], dtype=np.float32)
    gcols = np.concatenate([gamma, p0, extra]).astype(np.float32)  # [128]
    g_arr = np.ascontiguousarray(np.broadcast_to(gcols[None, :], (P, 2 * N_STEP + 2)))

    x = np.asarray(x, dtype=np.float32)
    xT = x.T  # [64, N_ROWS] view
    in_maps = []
    for k in range(N_CORES):
        sl = xT[:, k * RC : min((k + 1) * RC, N_ROWS)]
        if sl.shape[1] < RC:
            pad = np.zeros((N_COL, RC), np.float32)
            pad[:, : sl.shape[1]] = sl
            sl = pad
        else:
            sl = np.ascontiguousarray(sl)
        in_maps.append({"xt": sl, "g": g_arr})

    res = run_bass_kernel_spmd(
        nc, in_maps, list(range(N_CORES)), trace=TRACE
    )
    LAST["exec_time_ns"] = getattr(res, "exec_time_ns", None)
    LAST["profile_json"] = getattr(res, "profile_json", None)

    out = np.concatenate([res.results[k]["y"] for k in range(N_CORES)])
    return out[:N_ROWS].reshape(N_ROWS, 1)



# revision 35
# speedup vs baseline: 1.1667x; 1.1667x over previous
"""Trainium2 Bass kernel for nn_AggregationLayer (smooth and/or fold over 64 columns).

Math (default VARIANT="psum", the shift-softplus form):
  probs = softmax(selection_weights, axis=1)           # [63, 2]
  s_0 = x[:, 0]
  step i: s' = p0*smoothmin(s, x_i) + p1*smoothmax(s, x_i)   (sharpness 10)
Rewritten on the delta state D = s - x_next with per-step rescale abar
(c_i = max(p0,p1) >= 1/2, sigma_i = sign(p0-p1), abar_i = abar_{i-1}/c_i):
  D' = c*D + gamma*softplus(sigma*10*D) + Delta
  softplus(t) = ln(exp(t - C) + e^-C) + C     exactly, for |t| <= 65.7 <= C+44
(the Ln LUT is only accurate on inputs in [e^-44.6, e^44.6]; C=28 centers the
needed window). Scaled so the state coefficient is exactly 1:
  Dhat' = Dhat + gtil_i * L + dhat_i
  L = Ln(Exp(scale_i * Dhat))   # both on ACT; exp bias -C and the +gamma*C,
                                # exp-shift, and Delta terms all folded into
                                # the host-precomputed dhat_i rows.
Per step per stream: 2 ACT LUT ops (exp, ln) + 2 DVE ops (off-chain
tensor_add state+dhat, on-chain scalar_tensor_tensor gtil*L + tmp).
Two row-streams per core (F=492 each) keep ACT/DVE ~95% busy; the off-chain
add plus stream staggering hides the chain latency.

Distribution: pure data parallel over rows; 8 cores x 125,952 rows (last core
zero-padded). Host pre-transposes x, converts to scaled deltas, and folds all
per-step affine constants into the dhat rows (dh[0] is the initial state).
"""

import contextlib
import ctypes
import sys
import types

import numpy as np

P = 128          # SBUF partitions
F = 246          # rows per partition per tile
T = 4            # tiles per core
N_CORES = 8
RC = P * F * T   # 125,952 rows per core
N_ROWS = 1_000_000
N_COL = 64
N_STEP = 63

_CACHE = {}
TRACE = False
LAST = {}


# ---------------------------------------------------------------- axon NTFF shim
def _ensure_ntff_hook():
    """Provide antenv.axon_hooks (NTFF profiling) if the image lacks it."""
    try:
        from antenv.axon_hooks import get_axon_ntff_profile_hook  # noqa: F401
        return
    except ImportError:
        pass

    so_path = "/opt/axon/libaxon_pjrt.so"
    try:
        lib = ctypes.CDLL(so_path)
    except OSError:
        return
    if not hasattr(lib, "axon_start_nrt_profile"):
        return
    lib.axon_start_nrt_profile.argtypes = [ctypes.POINTER(ctypes.c_int64), ctypes.c_size_t]
    lib.axon_start_nrt_profile.restype = ctypes.c_int64
    lib.axon_stop_nrt_profile.argtypes = [ctypes.c_char_p]
    lib.axon_stop_nrt_profile.restype = ctypes.c_int64

    @contextlib.contextmanager
    def _hook(output_dir, device_ids):
        import jax

        jax.devices()
        if device_ids:
            ids = (ctypes.c_int64 * len(device_ids))(*device_ids)
            rc = lib.axon_start_nrt_profile(ids, len(device_ids))
        else:
            rc = lib.axon_start_nrt_profile(None, 0)
        if rc != 0:
            raise RuntimeError(f"axon_start_nrt_profile rc={rc}")
        try:
            yield
        finally:
            n = lib.axon_stop_nrt_profile(str(output_dir).encode())
            print(f"profile: {n} file(s) written to {output_dir}", file=sys.stderr)

    mod = types.ModuleType("antenv.axon_hooks")
    mod.get_axon_ntff_profile_hook = lambda: _hook
    mod.set_axon_ntff_profile_hook = lambda h: None
    sys.modules["antenv.axon_hooks"] = mod


# ---------------------------------------------------------------- device program
def _patch_act_tables(bacc, mybir):
    """Pin exp/ln/abs to the one set containing all three.

    Bacc's act-table-load pass assigns each activation the FIRST set that
    contains its function (exp -> exp_and_others, ln -> natural_log), which
    makes exp..ln sequences reload tables on every step (~2.6us each).
    Removing those funcs from every set except natural_log_exp_and_others
    (indices preserved) forces a single resident table set.
    """
    if getattr(bacc, "_act_tables_patched", False):
        return
    import concourse.hw_specs as hw_specs

    orig = hw_specs.get_activation_tables
    AF = mybir.ActivationFunctionType
    pinned = {AF.Exp, AF.Ln, AF.Abs}

    def patched(module_arch):
        tables = dict(orig(module_arch))
        out = {}
        for name, funcs in tables.items():
            if name == "natural_log_exp_and_others":
                out[name] = funcs
            else:
                out[name] = funcs - pinned
        return out

    bacc.get_activation_tables = patched
    bacc._act_tables_patched = True


SHIFT_C = 28.0            # exp-domain shift: exp(10d - C), ln(. + e^-C) + C
                          # data max |10D| = 65.7; Ln LUT valid on [e^-44.6, e^44.6]
VARIANT = "psum"          # "delta" (baseline) or "psum" (PSUM-state shift)

# --- psum-variant knobs
import os as _os
PSUM_F = int(_os.environ.get("PSUM_F", 492))       # free dim per tile (F*T == 984)
PSUM_T = int(_os.environ.get("PSUM_T", 2))         # concurrent row streams per core
PSUM_COMBINE = _os.environ.get("PSUM_COMBINE", "dve2")  # "sttmm"|"mm2"|"mix"|"dve2"
PSUM_PREFETCH = int(_os.environ.get("PSUM_PREFETCH", 6))
TT_ENGINE = _os.environ.get("TT_ENGINE", "dve")      # off-chain add engine
SHIFT_C = float(_os.environ.get("SHIFT_C", SHIFT_C))
VARIANT = _os.environ.get("KVARIANT", VARIANT)


def _build_nc(F=F, T=T, variant=None, dma_chunk=8, skew=1, tmp_bufs=3, sp_bufs=4):
    import concourse.bacc as bacc
    import concourse.mybir as mybir
    import concourse.tile as tile
    import math

    _patch_act_tables(bacc, mybir)
    variant = variant or VARIANT

    AF = mybir.ActivationFunctionType
    OP = mybir.AluOpType
    f32 = mybir.dt.float32
    RC = P * F * T

    nc = bacc.Bacc(None)
    xT = nc.dram_tensor("xt", [N_COL, RC], f32, kind="ExternalInput")
    # consts: cols 0..62 = gamma_i, 63..125 = p0_i, 126 = -C, 127 = exp(-C)
    gD = nc.dram_tensor("g", [P, 2 * N_STEP + 2], f32, kind="ExternalInput")
    w2D = (nc.dram_tensor("w2", [P, 2 * P], f32, kind="ExternalInput")
           if variant == "pe" else None)
    outD = nc.dram_tensor("y", [RC], f32, kind="ExternalOutput")

    with tile.TileContext(nc) as tc:
        with (
            tc.tile_pool(name="xp", bufs=3) as xp,
            tc.tile_pool(name="tmp", bufs=tmp_bufs) as tmp,
            tc.tile_pool(name="sp", bufs=sp_bufs) as sp,
            tc.tile_pool(name="gp", bufs=1) as gp,
            tc.tile_pool(name="pp", bufs=4, space="PSUM") as pp,
        ):
            g_sb = gp.tile([P, 2 * N_STEP + 2], f32)
            nc.sync.dma_start(out=g_sb[:], in_=gD[:])
            if variant == "pe":
                w2_sb = gp.tile([P, 2 * P], f32)
                nc.sync.dma_start(out=w2_sb[:], in_=w2D[:])

            def gamma_ap(i):
                return g_sb[:, i - 1 : i]

            def p0_ap(i):
                return g_sb[:, N_STEP + i - 1 : N_STEP + i]

            def emit_step_safe(s_prev, xi, i):
                d = tmp.tile([P, F], f32, tag="d")
                nc.vector.tensor_sub(d[:], s_prev, xi)
                a = tmp.tile([P, F], f32, tag="a")
                nc.scalar.activation(a[:], d[:], AF.Abs, scale=-10.0)
                E = tmp.tile([P, F], f32, tag="E")
                nc.scalar.activation(E[:], a[:], AF.Exp, scale=-1.0)
                L = tmp.tile([P, F], f32, tag="L")
                nc.scalar.activation(L[:], E[:], AF.Ln, bias=1.0)
                u = tmp.tile([P, F], f32, tag="u")
                nc.vector.scalar_tensor_tensor(u[:], d[:], 0.5, xi, OP.mult, OP.add)
                w = tmp.tile([P, F], f32, tag="w")
                nc.vector.scalar_tensor_tensor(w[:], a[:], 0.5, L[:], OP.mult, OP.add)
                s_new = sp.tile([P, F], f32, tag="s")
                nc.vector.scalar_tensor_tensor(
                    s_new[:], w[:], gamma_ap(i), u[:], OP.mult, OP.add
                )
                return s_new[:]

            def emit_step_dvabs(s_prev, xi, i):
                # abs on DVE (tensor_scalar, 2x fp32) -> ACT chain is exp,ln only
                d = tmp.tile([P, F], f32, tag="d")
                nc.vector.tensor_sub(d[:], s_prev, xi)
                a = tmp.tile([P, F], f32, tag="a")
                i32 = mybir.dt.int32
                nc.vector.tensor_scalar(
                    out=a[:].bitcast(i32), in0=d[:].bitcast(i32),
                    scalar1=0x7FFFFFFF, scalar2=0, op0=OP.bitwise_and, op1=OP.bitwise_or,
                )  # a = |d| (sign bit cleared)
                E = tmp.tile([P, F], f32, tag="E")
                nc.scalar.activation(E[:], a[:], AF.Exp, scale=-10.0)
                L = tmp.tile([P, F], f32, tag="L")
                nc.scalar.activation(L[:], E[:], AF.Ln, bias=1.0)
                u = tmp.tile([P, F], f32, tag="u")
                nc.vector.scalar_tensor_tensor(u[:], d[:], 0.5, xi, OP.mult, OP.add)
                w = tmp.tile([P, F], f32, tag="w")
                nc.vector.scalar_tensor_tensor(w[:], a[:], 5.0, L[:], OP.mult, OP.add)
                s_new = sp.tile([P, F], f32, tag="s")
                nc.vector.scalar_tensor_tensor(
                    s_new[:], w[:], gamma_ap(i), u[:], OP.mult, OP.add
                )
                return s_new[:]

            from concourse.tile_rust import add_dep_helper

            delta_ops = {}  # tile id -> list of (chunk_idx, mybir inst)

            def emit_step_delta(s_prev, xi, i, abs_on_act=False, t=None):
                # state D = s - x_next; xi holds Delta_i (or x_63 on the last step)
                # D' = 0.5*D + 5*gamma*|D| + gamma*L(|D|) + Delta_i
                a = tmp.tile([P, F], f32, tag="a")
                if abs_on_act:
                    a_inst = nc.scalar.activation(a[:], s_prev, AF.Abs)  # a = |D|
                else:
                    i32 = mybir.dt.int32
                    a_inst = nc.vector.tensor_scalar(
                        out=a[:].bitcast(i32), in0=s_prev.bitcast(i32),
                        scalar1=0x7FFFFFFF, scalar2=0, op0=OP.bitwise_and, op1=OP.bitwise_or,
                    )  # a = |D|
                E = tmp.tile([P, F], f32, tag="E")
                nc.scalar.activation(E[:], a[:], AF.Exp, scale=-10.0)
                L = tmp.tile([P, F], f32, tag="L")
                nc.scalar.activation(L[:], E[:], AF.Ln, bias=1.0)
                u = tmp.tile([P, F], f32, tag="u")
                u_inst = nc.vector.scalar_tensor_tensor(
                    u[:], s_prev, 0.5, xi, OP.mult, OP.add
                )
                # The in-place x->Delta rewrite is invisible to Tile's dep
                # tracker (out aliases in0); pin the ordering explicitly.
                if t is not None and t in delta_ops:
                    for lo, hi, dinst in delta_ops[t]:
                        if i == 1 and lo == 0:
                            add_dep_helper(a_inst.ins, dinst,
                                           reason="step1 reads Delta_0 state")
                        if i < N_COL - 1 and lo <= i < hi:
                            add_dep_helper(u_inst.ins, dinst,
                                           reason="step reads Delta col")
                w = tmp.tile([P, F], f32, tag="w")
                nc.vector.scalar_tensor_tensor(w[:], a[:], 5.0, L[:], OP.mult, OP.add)
                s_new = sp.tile([P, F], f32, tag="s")
                nc.vector.scalar_tensor_tensor(
                    s_new[:], w[:], gamma_ap(i), u[:], OP.mult, OP.add
                )
                return s_new[:]

            def emit_step_shift(s_prev, xi, i):
                # sp(10d) = ln(exp(10d - C) + e^-C) + C  (exact; no abs needed)
                # s' = p0*d + x_i + gamma*sp(10d)
                d = tmp.tile([P, F], f32, tag="d")
                nc.vector.tensor_sub(d[:], s_prev, xi)
                E = tmp.tile([P, F], f32, tag="E")
                nc.scalar.activation(
                    E[:], d[:], AF.Exp, scale=10.0, bias=g_sb[:, 126:127]
                )
                L = tmp.tile([P, F], f32, tag="L")
                nc.scalar.activation(L[:], E[:], AF.Ln, bias=g_sb[:, 127:128])
                u = tmp.tile([P, F], f32, tag="u")
                nc.vector.scalar_tensor_tensor(u[:], d[:], p0_ap(i), xi, OP.mult, OP.add)
                Lp = tmp.tile([P, F], f32, tag="Lp")
                nc.vector.tensor_scalar(
                    out=Lp[:], in0=L[:], scalar1=SHIFT_C, scalar2=gamma_ap(i),
                    op0=OP.add, op1=OP.mult,
                )
                s_new = sp.tile([P, F], f32, tag="s")
                nc.vector.tensor_add(s_new[:], Lp[:], u[:])
                return s_new[:]

            def emit_step_pe(s_prev, xi, i):
                # u = 0.5*D + Delta_i on the TensorEngine (identity matmuls
                # accumulating in PSUM); DVE does a, w, s' only.
                a = tmp.tile([P, F], f32, tag="a")
                i32 = mybir.dt.int32
                nc.vector.tensor_scalar(
                    out=a[:].bitcast(i32), in0=s_prev.bitcast(i32),
                    scalar1=0x7FFFFFFF, scalar2=0, op0=OP.bitwise_and, op1=OP.bitwise_or,
                )
                E = tmp.tile([P, F], f32, tag="E")
                nc.scalar.activation(E[:], a[:], AF.Exp, scale=-10.0)
                L = tmp.tile([P, F], f32, tag="L")
                nc.scalar.activation(L[:], E[:], AF.Ln, bias=1.0)
                u_ps = pp.tile([P, F], f32, tag="ups")
                nc.tensor.matmul(u_ps[:], w2_sb[:, 0:P], s_prev, start=True, stop=False)
                nc.tensor.matmul(u_ps[:], w2_sb[:, P : 2 * P], xi, start=False, stop=True)
                w = tmp.tile([P, F], f32, tag="w")
                nc.vector.scalar_tensor_tensor(w[:], a[:], 5.0, L[:], OP.mult, OP.add)
                s_new = sp.tile([P, F], f32, tag="s")
                nc.vector.scalar_tensor_tensor(
                    s_new[:], w[:], gamma_ap(i), u_ps[:], OP.mult, OP.add
                )
                return s_new[:]

            emit_step = {"shift": emit_step_shift, "safe": emit_step_safe,
                         "dvabs": emit_step_dvabs, "delta": emit_step_delta,
                         "delta_actabs": (lambda s_, x_, i_, t=None: emit_step_delta(s_, x_, i_, True, t=t)),
                         "delta_hyb": (lambda s_, x_, i_, t=None: emit_step_delta(s_, x_, i_, (t is not None and t % 2 == 0), t=t)),
                         "pe": emit_step_pe}[variant]

            DMA_CHUNK = dma_chunk
            SKEW = skew

            def emit_out(t, s_fin):
                dst = outD[t * P * F : (t + 1) * P * F].rearrange("(p j) -> p j", p=P)
                nc.sync.dma_start(out=dst, in_=s_fin)

            for pair in range(T // 2):
                tiles = [2 * pair, 2 * pair + 1]
                xt = {}
                for t in tiles:
                    xt[t] = xp.tile([P, N_COL, F], f32, tag="xt", name=f"xt{t}")
                    base = t * P * F
                    for c in range(0, N_COL, DMA_CHUNK):
                        src = xT[
                            c : c + DMA_CHUNK, base : base + P * F
                        ].rearrange("i (p j) -> p i j", p=P)
                        nc.sync.dma_start(out=xt[t][:, c : c + DMA_CHUNK, :], in_=src)
                    if variant.startswith("delta"):
                        # in-place x -> Delta: x[:, i, :] -= x[:, i+1, :] for i<63.
                        # Writes trail all reads of each position (reads of pos p
                        # occur at elements <= p*F, the write at p*F + pipe lat).
                        delta_ops[t] = []
                        for c in range(0, N_STEP, DMA_CHUNK):
                            hi = min(c + DMA_CHUNK, N_STEP)
                            dinst = nc.vector.tensor_sub(
                                xt[t][:, c:hi, :],
                                xt[t][:, c:hi, :],
                                xt[t][:, c + 1 : hi + 1, :],
                            )
                            delta_ops[t].append((c, hi, dinst.ins))

                tA, tB = tiles
                s_ap = {t: xt[t][:, 0, :] for t in tiles}
                for i in range(1, N_STEP + SKEW + 2):
                    if i <= N_STEP:
                        s_ap[tA] = emit_step(s_ap[tA], xt[tA][:, i, :], i,
                                             **({"t": tA} if variant.startswith("delta") else {}))
                    elif i == N_STEP + 1:
                        emit_out(tA, s_ap[tA])
                    j = i - SKEW
                    if 1 <= j <= N_STEP:
                        s_ap[tB] = emit_step(s_ap[tB], xt[tB][:, j, :], j,
                                             **({"t": tB} if variant.startswith("delta") else {}))
                    elif j == N_STEP + 1:
                        emit_out(tB, s_ap[tB])

    nc.finalize()
    return nc


def _build_nc_psum(F=None, T=None, combine=None, prefetch=None):
    """PSUM-state kernel: per step i (1..63)
         E = Exp(scale_i * Dhat)          [ACT, psum -> psum, no bias]
         L = Ln(E + e^-C)                 [ACT, psum -> sbuf, const-col bias]
         t = gtil_i * L + dhat_i          [DVE stt]  (sttmm)
         Dhat += t                        [PE eye-matmul accumulate]
       State Dhat lives in PSUM; all per-step affine constants folded into
       host-precomputed dhat rows (delta + gamma*C + exp-shift terms)."""
    import concourse.bacc as bacc
    import concourse.mybir as mybir
    import concourse.tile as tile

    F = F or PSUM_F
    T = T or PSUM_T
    combine = combine or PSUM_COMBINE
    prefetch = prefetch or PSUM_PREFETCH

    _patch_act_tables(bacc, mybir)

    AF = mybir.ActivationFunctionType
    OP = mybir.AluOpType
    f32 = mybir.dt.float32
    RC = P * F * T

    nc = bacc.Bacc(None)
    xT = nc.dram_tensor("xt", [N_COL, RC], f32, kind="ExternalInput")
    # g cols: 0 = e^-C (ln bias), 1 = 1/abar63, 2+i = gtil_{i+1} (i=0..62)
    gD = nc.dram_tensor("g", [P, 2 + N_STEP], f32, kind="ExternalInput")
    # w cols: [eye | gtil_1*eye | ... | gtil_63*eye] (only eye used by sttmm)
    NW = P * (1 + (N_STEP if combine in ("mm2", "mix") else 0))
    wD = nc.dram_tensor("w", [P, NW], f32, kind="ExternalInput")
    outD = nc.dram_tensor("y", [RC], f32, kind="ExternalOutput")

    # host fills via _CACHE["psum_consts"]; scales are python floats baked
    # into the instruction stream, so the build needs them up front.
    scales = _CACHE["psum_scales"]  # [63] python floats, step i -> scales[i-1]
    gtils = _CACHE["psum_gtil"]     # [63] python floats (stt immediates)
    inv_a63 = _CACHE["psum_inv_a63"]

    with tile.TileContext(nc) as tc:
        with (
            tc.tile_pool(name="xp", bufs=prefetch) as xp,
            tc.tile_pool(name="lp", bufs=3) as lp,
            tc.tile_pool(name="tp", bufs=3) as tp,
            tc.tile_pool(name="sp2", bufs=3) as sp2,
            tc.tile_pool(name="op", bufs=2) as op_,
            tc.tile_pool(name="gp", bufs=1) as gp,
            tc.tile_pool(name="stp", bufs=1, space="PSUM") as stp,
            tc.tile_pool(name="ep", bufs=2) as ep,
        ):
            def fetch(t, i):
                tl = xp.tile([P, F], f32, tag=f"x{t}", name=f"x{t}_{i}")
                base = t * P * F
                src = xT[i, base : base + P * F].rearrange("(p j) -> p j", p=P)
                # split the ramp burst across both HWDGE queues; ACT's
                # queue is idle until the pipeline fills
                eng = nc.scalar if (t % 2 == 1 and i <= prefetch) else nc.sync
                eng.dma_start(out=tl[:], in_=src)
                return tl

            g_sb = gp.tile([P, 2 + N_STEP], f32)
            w_sb = gp.tile([P, NW], f32)

            def gw(i):  # gtil_i * eye   (mm2 only)
                return w_sb[:, i * P : (i + 1) * P]

            def gcol(j):  # gtil for step j+1 as [P,1] (g cols 2..64)
                return g_sb[:, 2 + j : 3 + j]

            # stream modes: "dve" = SBUF state, DVE combine (tt off-chain +
            # stt on-chain); "mm2"/"sttmm" = PSUM state, PE accumulate.
            if combine == "mix":
                modes = ["dve" if t % 2 == 0 else "mm2" for t in range(T)]
            elif combine == "dve2":
                modes = ["dve"] * T
            else:
                modes = [combine] * T

            # x0 fetches first (they gate the first exp), then consts;
            # the eye/weight tile is only needed when the PE is used
            x0s = {t: fetch(t, 0) for t in range(T)}
            nc.sync.dma_start(out=g_sb[:], in_=gD[:])
            eye = None
            if any(m != "dve" for m in modes):
                nc.sync.dma_start(out=w_sb[:], in_=wD[:])
                eye = w_sb[:, 0:P]

            state = {}
            for t in range(T):
                if modes[t] == "dve":
                    state[t] = x0s[t][:]  # Δ̂ row 0 IS the initial state
                else:
                    st = stp.tile([P, F], f32, name=f"st{t}")
                    nc.tensor.matmul(st[:], eye, x0s[t][:], start=True,
                                     stop=True, skip_group_check=True)
                    state[t] = st[:]

            for i in range(1, N_STEP + 1):
                for t in range(T):
                    E = ep.tile([P, F], f32, tag=f"E{t}")
                    nc.scalar.activation(E[:], state[t], AF.Exp,
                                         scale=float(scales[i - 1]))
                    xti = fetch(t, i)
                    if modes[t] == "dve":
                        # off-chain: tmp = state + Δ̂_i (pure read of state)
                        if TT_ENGINE == "dma":
                            # in-place inline add on the fetched Δ̂ tile
                            # (accum DMA requires the SWDGE/gpsimd path)
                            nc.gpsimd.dma_start(out=xti[:], in_=state[t],
                                                accum_op=OP.add)
                            tmp = xti
                        else:
                            tmp = tp.tile([P, F], f32, tag=f"tmp{t}")
                            eng = nc.gpsimd if TT_ENGINE == "gpsimd" else nc.vector
                            eng.tensor_add(tmp[:], state[t], xti[:])
                    elif modes[t] == "mm2":
                        nc.tensor.matmul(state[t], eye, xti[:], start=False,
                                         stop=True, skip_group_check=True)
                    L = lp.tile([P, F], f32, tag=f"L{t}")
                    nc.scalar.activation(L[:], E[:], AF.Ln, bias=g_sb[:, 0:1])
                    if modes[t] == "dve":
                        ns = sp2.tile([P, F], f32, tag=f"s{t}")
                        nc.vector.scalar_tensor_tensor(
                            ns[:], L[:], float(gtils[i - 1]), tmp[:],
                            OP.mult, OP.add
                        )
                        state[t] = ns[:]
                    elif modes[t] == "mm2":
                        nc.tensor.matmul(state[t], gw(i), L[:], start=False,
                                         stop=True, skip_group_check=True)
                    else:  # sttmm
                        tt = tp.tile([P, F], f32, tag=f"t{t}")
                        nc.vector.scalar_tensor_tensor(
                            tt[:], L[:], gcol(i - 1), xti[:], OP.mult, OP.add
                        )
                        nc.tensor.matmul(state[t], eye, tt[:], start=False,
                                         stop=True, skip_group_check=True)

            for t in range(T):
                dst = outD[t * P * F : (t + 1) * P * F].rearrange(
                    "(p j) -> p j", p=P)
                if abs(inv_a63 - 1.0) < 1e-6:
                    nc.sync.dma_start(out=dst, in_=state[t])
                else:
                    o = op_.tile([P, F], f32, tag="o")
                    nc.vector.tensor_scalar(out=o[:], in0=state[t],
                                            scalar1=float(inv_a63),
                                            scalar2=None, op0=OP.mult)
                    nc.sync.dma_start(out=dst, in_=o[:])

    nc.finalize()
    return nc


def _psum_host_prep(x, selection_weights, F, T):
    """Fold softmax, state rescaling, exp-shift and gamma*C constants into
    per-step dhat rows; returns (xt_shards, g_arr, w_arr, inv_a63)."""
    RC = P * F * T
    C = SHIFT_C
    w64 = selection_weights.astype(np.float64)
    e = np.exp(w64 - w64.max(axis=1, keepdims=True))
    p = e / e.sum(axis=1, keepdims=True)
    p0, p1 = p[:, 0], p[:, 1]          # [63], step i uses index i-1
    gamma = (p1 - p0) / 10.0
    c = np.maximum(p0, p1)
    sigma = np.where(p0 >= p1, 1.0, -1.0)

    abar = np.empty(64, dtype=np.float64)
    abar[0] = np.prod(c)               # => abar[63] == 1: state IS the output
    for i in range(1, 64):
        abar[i] = abar[i - 1] / c[i - 1]

    scales = sigma * 10.0 / abar[:63]              # step i -> scales[i-1]
    gtil = abar[1:] * gamma                        # step i -> gtil[i-1]
    shift = np.zeros(65, dtype=np.float64)         # shift[i], i=1..63; 64 -> 0
    shift[1:64] = sigma * abar[:63] * C / 10.0

    _CACHE["psum_scales"] = [float(s) for s in scales]
    _CACHE["psum_gtil"] = [float(g) for g in gtil]
    _CACHE["psum_inv_a63"] = float(1.0 / abar[63])

    # dhat rows on the transposed view
    x = np.asarray(x, dtype=np.float32)
    xT = x.T                                        # [64, N] view
    dh = np.empty((N_COL, N_ROWS), dtype=np.float32)
    # row 0: abar0*(x0-x1) - shift_1
    dh[0] = (xT[0] - xT[1]) * np.float32(abar[0])
    dh[0] += np.float32(-shift[1])
    for i in range(1, 63):
        np.subtract(xT[i], xT[i + 1], out=dh[i])
        dh[i] *= np.float32(abar[i])
        dh[i] += np.float32(abar[i] * gamma[i - 1] * C + shift[i] - shift[i + 1])
    dh[63] = xT[63] * np.float32(abar[63])
    dh[63] += np.float32(abar[63] * gamma[62] * C + shift[63])

    gcols = np.zeros(2 + N_STEP, dtype=np.float32)
    gcols[0] = np.exp(-C)
    gcols[1] = 1.0 / abar[63]
    gcols[2:] = gtil.astype(np.float32)
    g_arr = np.ascontiguousarray(np.broadcast_to(gcols[None, :], (P, 2 + N_STEP)))

    if PSUM_COMBINE in ("mm2", "mix"):
        w_arr = np.zeros((P, P * 64), dtype=np.float32)
        w_arr[:, :P] = np.eye(P, dtype=np.float32)
        for i in range(1, 64):
            w_arr[:, i * P : (i + 1) * P] = np.eye(P) * np.float32(gtil[i - 1])
    else:
        w_arr = np.ascontiguousarray(np.eye(P, dtype=np.float32))

    shards = []
    for k in range(N_CORES):
        sl = dh[:, k * RC : min((k + 1) * RC, N_ROWS)]
        if sl.shape[1] < RC:
            pad = np.zeros((N_COL, RC), np.float32)
            pad[:, : sl.shape[1]] = sl
            sl = pad
        else:
            sl = np.ascontiguousarray(sl)
        shards.append(sl)
    return shards, g_arr, w_arr


def _get_nc():
    if "nc" not in _CACHE:
        _CACHE["nc"] = _build_nc()
    return _CACHE["nc"]


# ---------------------------------------------------------------- host wrapper
def _kernel_psum(x: np.ndarray, selection_weights: np.ndarray) -> np.ndarray:
    _ensure_ntff_hook()
    from concourse.bass_utils import run_bass_kernel_spmd

    F, T = PSUM_F, PSUM_T
    RC = P * F * T
    shards, g_arr, w_arr = _psum_host_prep(x, selection_weights, F, T)
    key = ("psum", F, T, PSUM_COMBINE)
    if _CACHE.get("nc_key") != key:
        _CACHE["nc_psum"] = _build_nc_psum(F=F, T=T)
        _CACHE["nc_key"] = key
    nc = _CACHE["nc_psum"]

    in_maps = [{"xt": shards[k], "g": g_arr, "w": w_arr} for k in range(N_CORES)]
    res = run_bass_kernel_spmd(nc, in_maps, list(range(N_CORES)), trace=TRACE)
    LAST["exec_time_ns"] = getattr(res, "exec_time_ns", None)
    LAST["profile_json"] = getattr(res, "profile_json", None)

    out = np.concatenate([res.results[k]["y"] for k in range(N_CORES)])
    return out[:N_ROWS].reshape(N_ROWS, 1)


def kernel(x: np.ndarray, selection_weights: np.ndarray) -> np.ndarray:
    if VARIANT == "psum":
        return _kernel_psum(x, selection_weights)
    _ensure_ntff_hook()
    from concourse.bass_utils import run_bass_kernel_spmd

    nc = _get_nc()

    # softmax over the (and, or) pair, in float64 for clean constants
    w64 = selection_weights.astype(np.float64)
    e = np.exp(w64 - w64.max(axis=1, keepdims=True))
    p = e / e.sum(axis=1, keepdims=True)
    gamma = ((p[:, 1] - p[:, 0]) / 10.0).astype(np.float32)  # [63]
    p0 = p[:, 0].astype(np.float32)                          # [63]
    extra = np.array([-SHIFT_C, np.exp(-SHIFT_C)], dtype=np.float32)
    gcols = np.concatenate([gamma, p0, extra]).astype(np.float32)  # [128]
    g_arr = np.ascontiguousarray(np.broadcast_to(gcols[None, :], (P, 2 * N_STEP + 2)))

    x = np.asarray(x, dtype=np.float32)
    xT = x.T  # [64, N_ROWS] view
    in_maps = []
    for k in range(N_CORES):
        sl = xT[:, k * RC : min((k + 1) * RC, N_ROWS)]
        if sl.shape[1] < RC:
            pad = np.zeros((N_COL, RC), np.float32)
            pad[:, : sl.shape[1]] = sl
            sl = pad
        else:
            sl = np.ascontiguousarray(sl)
        in_maps.append({"xt": sl, "g": g_arr})

    res = run_bass_kernel_spmd(
        nc, in_maps, list(range(N_CORES)), trace=TRACE
    )
    LAST["exec_time_ns"] = getattr(res, "exec_time_ns", None)
    LAST["profile_json"] = getattr(res, "profile_json", None)

    out = np.concatenate([res.results[k]["y"] for k in range(N_CORES)])
    return out[:N_ROWS].reshape(N_ROWS, 1)

